# revision 29
# baseline (speedup 1.0000x reference)
"""Trainium2 Bass kernel: out = input * diag (elementwise column scale).

input  : (4, 4096, 4096) f32
diag   : (4096,)          f32
output : (4, 4096, 4096) f32

Strategy: data-parallel over 8 NeuronCores. Flatten input rows
(4*4096 = 16384) and give each core 2048 rows x 4096 cols (32 MiB).
diag is replicated to every core. Each core streams its shard through
SBUF in big tiles (128 partitions x F_TILE f32), multiplies by a
partition-broadcast copy of diag on the vector engine, and streams the
result back to DRAM. Pure memory-bound streaming: ~64 MiB of HBM
traffic per core at the ~426 GB/s measured per-core DMA rate ->
~160 us streaming floor; measured ~170 us end to end (bit-exact).
"""

import time

import numpy as np

import concourse.bacc as bacc
import concourse.tile as tile
from concourse import mybir
from concourse.bass_utils import run_bass_kernel_spmd

N_CORES = 8
B, S, D = 4, 4096, 4096
ROWS = B * S                  # 16384
RPC = ROWS // N_CORES         # 2048 rows per core
P = 128                       # SBUF partitions

F_TILE = 16384                # free elems per partition per tile (64 KiB)
BUFS = 2

_cache = {}


def build(
    rpc=RPC,
    f_tile=F_TILE,
    bufs=BUFS,
    partition_id=False,
    store_engine="sync",
    diag_engine="sync",
    split_store=False,
    diag_pb=True,
    strip_preamble=True,
    late_diag=False,
):
    """Build + compile the per-core Bass program.

    Per core: x [rpc, D] -> y [rpc, D], both viewed as [128, rpc*D/128]
    so each partition line is a contiguous DRAM run. Every D-aligned
    segment of a partition line is one full row of the original matrix,
    so multiplying by diag (broadcast to all partitions) is exact
    regardless of which rows land where.
    """
    f32 = mybir.dt.float32
    nc = bacc.Bacc(
        "TRN2",
        target_bir_lowering=False,
        debug=False,
        num_devices=N_CORES,
        enable_partition_id=partition_id,
    )
    if strip_preamble:
        # Drop the constructor-emitted const-pool memsets and the start
        # all-engine barrier: this kernel never reads the const APs, and
        # TileContext's own entry barrier provides the cross-engine sync.
        # Keeps the per-engine register setup that precedes them.
        insts = nc.m.functions[0].blocks[0].instructions
        start = None
        for k, i in enumerate(insts):
            if type(i).__name__ == "InstMemset" and "const-" in str(i):
                start = k
                break
        if start is not None:
            end = start
            while end < len(insts) and type(insts[end]).__name__ in (
                "InstMemset",
                "InstDrain",
                "InstEventSemaphore",
            ):
                end += 1
            del insts[start:end]

    x = nc.dram_tensor("x", [rpc, D], f32, kind="ExternalInput").ap()
    dg = nc.dram_tensor("diag", [D], f32, kind="ExternalInput").ap()
    y = nc.dram_tensor("y", [rpc, D], f32, kind="ExternalOutput").ap()

    free = rpc * D // P
    assert free % f_tile == 0 and f_tile % D == 0
    reps = f_tile // D
    n_tiles = free // f_tile
    xv = x.rearrange("(p r) d -> p (r d)", p=P)
    yv = y.rearrange("(p r) d -> p (r d)", p=P)

    store_eng = {"sync": nc.sync, "scalar": nc.scalar}[store_engine]

    with tile.TileContext(nc) as tc:
        with (
            tc.tile_pool(name="dpool", bufs=1) as dpool,
            tc.tile_pool(name="work", bufs=bufs) as pool,
        ):
            dtile = dpool.tile([P, D], f32)
            diag_eng = {
                "sync": nc.sync,
                "scalar": nc.scalar,
                "gpsimd": nc.gpsimd,
            }[diag_engine]

            def emit_diag():
                if diag_pb:
                    # 16 KiB HBM read into partition 0, then an on-chip SWDGE
                    # partition broadcast: keeps the 2 MiB replication off HBM.
                    diag_eng.dma_start(dtile[0:1, :], dg[None, :])
                    nc.gpsimd.partition_broadcast(dtile[:], dtile[0:1, :])
                else:
                    # Stride-0 DRAM source: DMA reads the same 16 KiB 128x.
                    diag_eng.dma_start(dtile[:], dg[None, :].to_broadcast((P, D)))

            if not late_diag:
                emit_diag()
            for i in range(n_tiles):
                t = pool.tile([P, f_tile], f32)
                nc.sync.dma_start(t[:], xv[:, i * f_tile:(i + 1) * f_tile])
                if late_diag and i == 0:
                    # Issue the first x load ahead of the diag read so the
                    # 16 KiB diag DMA doesn't delay the stream start (~0.7 us
                    # SP issue latency per DMA). The broadcast still finishes
                    # long before the first mul needs it.
                    emit_diag()
                for j in range(reps):
                    sl = t[:, j * D:(j + 1) * D]
                    nc.vector.tensor_mul(sl, sl, dtile[:])
                    if split_store:
                        # Store each D-slice right after its mul: the store
                        # stream starts ~reps x earlier per tile.
                        store_eng.dma_start(
                            yv[:, i * f_tile + j * D:i * f_tile + (j + 1) * D],
                            sl,
                        )
                if not split_store:
                    store_eng.dma_start(yv[:, i * f_tile:(i + 1) * f_tile], t[:])
    nc.compile()
    return nc


def get_nc():
    key = (RPC, F_TILE, BUFS)
    if key not in _cache:
        _cache[key] = build(*key)
    return _cache[key]


def kernel(input, diag):
    x = np.ascontiguousarray(np.asarray(input, dtype=np.float32)).reshape(ROWS, D)
    dg = np.ascontiguousarray(np.asarray(diag, dtype=np.float32))
    nc = get_nc()
    shards = x.reshape(N_CORES, RPC, D)
    in_maps = [{"x": shards[c], "diag": dg} for c in range(N_CORES)]
    last_err = None
    for attempt in range(3):
        try:
            res = run_bass_kernel_spmd(nc, in_maps, list(range(N_CORES))).results
            break
        except Exception as e:  # transient device wedges (NRT_EXEC_UNIT_...)
            last_err = e
            try:
                import jax

                jax.clear_backends()
            except Exception:
                pass
            time.sleep(2.0)
    else:
        raise last_err
    out = np.concatenate([res[c]["y"] for c in range(N_CORES)], axis=0)
    return out.reshape(B, S, D)
